# revision 10
# baseline (speedup 1.0000x reference)
"""Trainium2 Bass kernel for the DQN hypergraph-conv network (8-core SPMD).

Sharding: edges row-sharded for the message stage (Hs@X@theta local per
edge shard), nodes column-sharded for the aggregation stage (Ht.T @ ...),
with AllGather collectives moving the small [E,H]/[N,H] intermediates.
The big Ht/Hs shards are read once in bf16 and stay resident in SBUF
across both conv layers.

Per core c (NCORES=8):
  hsT = Hs[e_c, :].T   [N, E/8]  bf16   (stage-1 moving operand)
  ht  = Ht[:, n_c]     [E, N/8]  bf16   (stage-2 moving operand)
  stage1: tmpT[f,e] = sum_n X[n-tile].T @ hsT[n-tile]      (PE, N=512 free)
  msg[e,h] = tmpT.T @ theta ; scaled = edge_w * msg        -> AllGather
  stage2: aggT[h,n] = sum_e scaled[e-tile].T @ ht[e-tile]
          + w_trans.T @ xiT + bias (rank-1)                (PE, N=512 free)
  epilogue: fused leaky-relu / dropout mask / second lrelu  (DVE)
  conv0 only: PE-transpose X1T -> X1 tiles -> AllGather for conv1 lhsT
  fc: fc_w.T @ XT (f32 matmul) + host-precomputed state term -> [1, N/8]
"""

import sys

if "/opt/trn_rl_repo" not in sys.path:
    sys.path.insert(0, "/opt/trn_rl_repo")

import numpy as np
import ml_dtypes

NCORES = 8
N, E, F = 8192, 4096, 128
E_SH = E // NCORES   # 512 edges per core
N_SH = N // NCORES   # 1024 nodes per core
NEG_SLOPE = 0.01
DROP_P = 0.5

_CACHE = {}


def _build_nc():
    import concourse.bacc as bacc
    import concourse.mybir as mybir
    import concourse.tile as tile

    bf16 = mybir.dt.bfloat16
    f32 = mybir.dt.float32
    Alu = mybir.AluOpType

    nc = bacc.Bacc("TRN2", target_bir_lowering=False, debug=False,
                   num_devices=NCORES)

    # ---- I/O ----
    hsT_d = nc.dram_tensor("hsT", [N, E_SH], bf16, kind="ExternalInput")
    ht_d = nc.dram_tensor("ht", [E, N_SH], bf16, kind="ExternalInput")
    xbf_d = nc.dram_tensor("xbf", [N, F], bf16, kind="ExternalInput")
    xiT_d = nc.dram_tensor("xiT", [F, N_SH], bf16, kind="ExternalInput")
    m2T_d = nc.dram_tensor("m2T", [F, N_SH], f32, kind="ExternalInput")
    ew_d = [nc.dram_tensor(f"ew{i}", [E_SH, 1], f32, kind="ExternalInput")
            for i in range(2)]
    th_d = [nc.dram_tensor(f"th{i}", [F, F], bf16, kind="ExternalInput")
            for i in range(2)]
    wt_d = [nc.dram_tensor(f"wt{i}", [F, F], bf16, kind="ExternalInput")
            for i in range(2)]
    b_d = [nc.dram_tensor(f"b{i}", [1, F], bf16, kind="ExternalInput")
           for i in range(2)]
    idn_d = nc.dram_tensor("idn", [F, F], bf16, kind="ExternalInput")
    fcw_d = nc.dram_tensor("fcw", [F, 1], f32, kind="ExternalInput")
    stT_d = nc.dram_tensor("stT", [1, N_SH], f32, kind="ExternalInput")
    out_d = nc.dram_tensor("out", [1, N_SH], f32, kind="ExternalOutput")

    RG = [list(range(NCORES))]

    with tile.TileContext(nc) as tc:
        with (
            tc.tile_pool(name="sb", bufs=1) as sb,
            tc.tile_pool(name="sc2", bufs=2) as sc2,
            tc.tile_pool(name="ps_tmp", bufs=1, space="PSUM") as ps_tmp,
            tc.tile_pool(name="ps_agg", bufs=2, space="PSUM") as ps_agg,
            tc.tile_pool(name="ps_sm", bufs=2, space="PSUM") as ps_sm,
            tc.tile_pool(name="dram", bufs=1, space="DRAM") as dram,
        ):
            # ---- parameter tiles ----
            th_sb = []
            wt_sb = []
            b_sb = []
            ew_sb = []
            for i in range(2):
                t = sb.tile([F, F], bf16, tag=f"th{i}")
                nc.sync.dma_start(t[:], th_d[i][:, :])
                th_sb.append(t)
                t = sb.tile([F, F], bf16, tag=f"wt{i}")
                nc.sync.dma_start(t[:], wt_d[i][:, :])
                wt_sb.append(t)
                t = sb.tile([1, F], bf16, tag=f"b{i}")
                nc.sync.dma_start(t[:], b_d[i][:, :])
                b_sb.append(t)
                t = sb.tile([128, 4, 1], f32, tag=f"ew{i}")
                nc.sync.dma_start(
                    t[:], ew_d[i].ap().rearrange("(c p) o -> p c o", p=128))
                ew_sb.append(t)
            idn_sb = sb.tile([F, F], bf16, tag="idn")
            nc.sync.dma_start(idn_sb[:], idn_d[:, :])
            xiT_sb = sb.tile([F, N_SH], bf16, tag="xiT")
            nc.sync.dma_start(xiT_sb[:], xiT_d[:, :])
            m2T_sb = sb.tile([F, N_SH], f32, tag="m2T")
            nc.sync.dma_start(m2T_sb[:], m2T_d[:, :])
            fcw_sb = sb.tile([F, 1], f32, tag="fcw")
            nc.sync.dma_start(fcw_sb[:], fcw_d[:, :])
            stT_sb = sb.tile([1, N_SH], f32, tag="stT")
            nc.sync.dma_start(stT_sb[:], stT_d[:, :])
            ones_sb = sb.tile([1, 512], bf16, tag="ones")
            nc.vector.memset(ones_sb[:], 1.0)

            # ---- big resident loads ----
            # x (conv0 stage-1 lhsT): 8 chunks of [128, 8, 128]
            xall = []
            x_r = xbf_d.ap().rearrange("(b p) h -> p b h", p=128)
            for i in range(8):
                t = sb.tile([128, 8, F], bf16, tag=f"x{i}")
                nc.sync.dma_start(t[:], x_r[:, i * 8:(i + 1) * 8, :])
                xall.append(t)
            # hsT: 16 chunks of [128, 4, 512]
            hsT_t = []
            hsT_r = hsT_d.ap().rearrange("(b p) e -> p b e", p=128)
            for i in range(16):
                t = sb.tile([128, 4, E_SH], bf16, tag=f"hsT{i}")
                nc.sync.dma_start(t[:], hsT_r[:, i * 4:(i + 1) * 4, :])
                hsT_t.append(t)

            # ht: 16 chunks of [128, 2, 1024] -- emitted later (after stage-1
            # matmuls) so the stage-1 streams win DMA-queue priority.
            ht_t = [None] * 16
            ht_r = ht_d.ap().rearrange("(b p) n -> p b n", p=128)

            # DRAM bounce buffers for collectives
            agm_in = [dram.tile([E_SH, F], bf16, tag=f"agmi{i}",
                                name=f"agmi{i}") for i in range(2)]
            agm_out = [dram.tile([E, F], bf16, addr_space="Shared",
                                 tag=f"agmo{i}", name=f"agmo{i}")
                       for i in range(2)]
            agx_in = dram.tile([N_SH, F], bf16, tag="agxi")
            agx_out = dram.tile([N, F], bf16, addr_space="Shared", tag="agxo")

            x1_t = [None] * 8    # conv1 stage-1 lhsT chunks
            xT = [None, None]    # final-layer activations (f32, [128,512] x2)

            for conv in range(2):
                # ---------- stage 1: tmpT = X.T @ HsT ----------
                tmpT_ps = ps_tmp.tile([128, E_SH], f32, tag="tmpT")
                for nt in range(64):
                    if conv == 0:
                        lhsT = xall[nt // 8][:, nt % 8, :]
                    else:
                        lhsT = x1_t[nt // 8][:, nt % 8, :]
                    nc.tensor.matmul(
                        tmpT_ps[:], lhsT, hsT_t[nt // 4][:, nt % 4, :],
                        start=(nt == 0), stop=(nt == 63))

                if conv == 0:
                    # schedule the ht loads now; they fill the DMA gap during
                    # the msg/AllGather phase
                    for i in range(16):
                        t = sb.tile([128, 2, N_SH], bf16, tag=f"ht{i}")
                        nc.sync.dma_start(t[:], ht_r[:, i * 2:(i + 1) * 2, :])
                        ht_t[i] = t

                tmpT_bf = sb.tile([128, E_SH], bf16, tag=f"tmpTbf{conv}")
                nc.vector.tensor_copy(tmpT_bf[:], tmpT_ps[:])

                # ---------- msg = tmpT.T @ theta, scaled by edge_w ----------
                for ec in range(4):
                    mps = ps_sm.tile([128, F], f32, tag="msg", bufs=2)
                    nc.tensor.matmul(
                        mps[:], tmpT_bf[:, ec * 128:(ec + 1) * 128],
                        th_sb[conv][:], start=True, stop=True)
                    mbf = sc2.tile([128, F], bf16, tag="mbf")
                    nc.vector.tensor_scalar(
                        mbf[:], mps[:], ew_sb[conv][:, ec, :], None, Alu.mult)
                    nc.sync.dma_start(
                        agm_in[conv][ec * 128:(ec + 1) * 128, :], mbf[:])

                nc.gpsimd.collective_compute(
                    "AllGather", Alu.bypass, replica_groups=RG,
                    ins=[agm_in[conv][:]], outs=[agm_out[conv][:]])

                # load gathered scaled msg: 4 chunks of [128, 8, 128]
                sc_t = []
                agm_r = agm_out[conv].rearrange("(b p) h -> p b h", p=128)
                for i in range(4):
                    t = sb.tile([128, 8, F], bf16, tag=f"sc{i}")
                    nc.sync.dma_start(t[:], agm_r[:, i * 8:(i + 1) * 8, :])
                    sc_t.append(t)

                # ---------- stage 2: aggT = scaled.T @ Ht + wT@xiT + b ----------
                for nb in range(2):
                    agg = ps_agg.tile([128, 512], f32, tag="agg")
                    for et in range(32):
                        nc.tensor.matmul(
                            agg[:], sc_t[et // 8][:, et % 8, :],
                            ht_t[et // 2][:, et % 2, nb * 512:(nb + 1) * 512],
                            start=(et == 0), stop=False)
                    nc.tensor.matmul(
                        agg[:], wt_sb[conv][:],
                        xiT_sb[:, nb * 512:(nb + 1) * 512],
                        start=False, stop=False)
                    nc.tensor.matmul(
                        agg[:], b_sb[conv][:], ones_sb[:],
                        start=False, stop=True)

                    if conv == 0:
                        # X1T = lrelu(agg) * dropout_mask   (bf16)
                        sl = sc2.tile([128, 512], f32, tag="sl")
                        nc.vector.tensor_scalar(
                            sl[:], agg[:], NEG_SLOPE, None, Alu.mult)
                        lr = sc2.tile([128, 512], f32, tag="lr")
                        nc.vector.tensor_tensor(lr[:], agg[:], sl[:], Alu.max)
                        x1t = sb.tile([128, 512], bf16, tag=f"x1t{nb}")
                        nc.vector.tensor_tensor(
                            x1t[:], lr[:], m2T_sb[:, nb * 512:(nb + 1) * 512],
                            Alu.mult)

                        # transpose X1T -> X1 node-major tiles, to DRAM for AG
                        for j in range(4):
                            ntile = nb * 4 + j
                            tps = ps_sm.tile([128, 128], bf16, tag="tr",
                                             bufs=2)
                            nc.tensor.transpose(
                                tps[:], x1t[:, j * 128:(j + 1) * 128],
                                idn_sb[:])
                            x1b = sc2.tile([128, 128], bf16, tag="x1b")
                            nc.vector.tensor_copy(x1b[:], tps[:])
                            nc.sync.dma_start(
                                agx_in[ntile * 128:(ntile + 1) * 128, :],
                                x1b[:])
                    else:
                        # X = lrelu(lrelu(agg)) = max(agg, 1e-4*agg)  (f32)
                        sl = sc2.tile([128, 512], f32, tag="sl")
                        nc.vector.tensor_scalar(
                            sl[:], agg[:], NEG_SLOPE * NEG_SLOPE, None,
                            Alu.mult)
                        t = sb.tile([128, 512], f32, tag=f"xT{nb}")
                        nc.vector.tensor_tensor(t[:], agg[:], sl[:], Alu.max)
                        xT[nb] = t

                if conv == 0:
                    nc.gpsimd.collective_compute(
                        "AllGather", Alu.bypass, replica_groups=RG,
                        ins=[agx_in[:]], outs=[agx_out[:]])
                    agx_r = agx_out.rearrange("(b p) h -> p b h", p=128)
                    for i in range(8):
                        # reuse the xall slots -- x is dead after conv0 stage 1
                        t = sb.tile([128, 8, F], bf16, tag=f"x{i}")
                        nc.sync.dma_start(t[:], agx_r[:, i * 8:(i + 1) * 8, :])
                        x1_t[i] = t

            # ---------- fc: out = fc_w[:128].T @ XT + (state*fc_w[128]+fc_b) ----------
            for nb in range(2):
                fps = ps_sm.tile([1, 512], f32, tag="fc", bufs=1)
                nc.tensor.matmul(fps[:], fcw_sb[:], xT[nb][:],
                                 start=True, stop=True)
                osb = sc2.tile([1, 512], f32, tag="osb")
                nc.vector.tensor_tensor(
                    osb[:], fps[:], stT_sb[:, nb * 512:(nb + 1) * 512],
                    Alu.add)
                nc.sync.dma_start(out_d[0:1, nb * 512:(nb + 1) * 512], osb[:])

    nc.compile()
    return nc


def _get_nc():
    if "nc" not in _CACHE:
        _CACHE["nc"] = _build_nc()
    return _CACHE["nc"]


def _dropout_mask2():
    """2.0 * bernoulli(key(42), 0.5, (N, F)) exactly as the reference."""
    import jax
    cpu = jax.devices("cpu")[0]
    with jax.default_device(cpu):
        keep = jax.random.bernoulli(jax.random.key(42), 1.0 - DROP_P, (N, F))
        return np.asarray(keep).astype(np.float32) * (1.0 / (1.0 - DROP_P))


def prepare_in_maps(xi, x, Ht, Hs, state,
                    w_trans0, theta0, edge_w0, bias0,
                    w_trans1, theta1, edge_w1, bias1,
                    fc_w, fc_b):
    bf = ml_dtypes.bfloat16
    mask2 = _dropout_mask2()

    xbf = np.ascontiguousarray(x, np.float32).astype(bf)
    idn = np.eye(F, dtype=bf)
    th = [np.asarray(theta0, np.float32).astype(bf),
          np.asarray(theta1, np.float32).astype(bf)]
    wt = [np.asarray(w_trans0, np.float32).astype(bf),
          np.asarray(w_trans1, np.float32).astype(bf)]
    b = [np.asarray(bias0, np.float32).reshape(1, F).astype(bf),
         np.asarray(bias1, np.float32).reshape(1, F).astype(bf)]
    ew = [np.asarray(edge_w0, np.float32).reshape(E_SH * NCORES, 1),
          np.asarray(edge_w1, np.float32).reshape(E_SH * NCORES, 1)]
    fcw = np.ascontiguousarray(np.asarray(fc_w, np.float32)[:F, :])  # [128,1]
    fcw_last = float(np.asarray(fc_w, np.float32)[F, 0])
    fcb = float(np.asarray(fc_b, np.float32)[0])

    Hs32 = np.asarray(Hs, np.float32)
    Ht32 = np.asarray(Ht, np.float32)
    xi32 = np.asarray(xi, np.float32)
    st32 = np.asarray(state, np.float32)

    in_maps = []
    for c in range(NCORES):
        e0, e1 = c * E_SH, (c + 1) * E_SH
        n0, n1 = c * N_SH, (c + 1) * N_SH
        stT = (st32[n0:n1, 0] * fcw_last + fcb).reshape(1, N_SH)
        in_maps.append({
            "hsT": np.ascontiguousarray(Hs32[e0:e1, :].T).astype(bf),
            "ht": np.ascontiguousarray(Ht32[:, n0:n1]).astype(bf),
            "xbf": xbf,
            "xiT": np.ascontiguousarray(xi32[n0:n1, :].T).astype(bf),
            "m2T": np.ascontiguousarray(mask2[n0:n1, :].T),
            "ew0": np.ascontiguousarray(ew[0][e0:e1]),
            "ew1": np.ascontiguousarray(ew[1][e0:e1]),
            "th0": th[0], "th1": th[1],
            "wt0": wt[0], "wt1": wt[1],
            "b0": b[0], "b1": b[1],
            "idn": idn,
            "fcw": fcw,
            "stT": np.ascontiguousarray(stT, np.float32),
        })
    return in_maps


def kernel(xi, x, Ht, Hs, state,
           w_trans0, theta0, edge_w0, bias0,
           w_trans1, theta1, edge_w1, bias1,
           fc_w, fc_b, _trace=False):
    from concourse.bass_utils import run_bass_kernel_spmd

    nc = _get_nc()
    in_maps = prepare_in_maps(
        xi, x, Ht, Hs, state,
        w_trans0, theta0, edge_w0, bias0,
        w_trans1, theta1, edge_w1, bias1,
        fc_w, fc_b)
    res = run_bass_kernel_spmd(
        nc, in_maps, core_ids=list(range(NCORES)), trace=_trace)
    if _trace:
        _CACHE["last_results"] = res
    out = np.concatenate(
        [res.results[c]["out"].reshape(N_SH) for c in range(NCORES)])
    return out.reshape(N, 1).astype(np.float32)


# revision 14
# speedup vs baseline: 1.1011x; 1.1011x over previous
"""Trainium2 Bass kernel for the DQN hypergraph-conv network (8-core SPMD).

Sharding: edges row-sharded for the message stage (Hs@X@theta local per
edge shard), nodes column-sharded for the aggregation stage (Ht.T @ ...),
with AllGather collectives moving the small [E,H]/[N,H] intermediates.
The big Ht/Hs shards are read once in bf16 and stay resident in SBUF
across both conv layers.

Per core c (NCORES=8):
  hsT = Hs[e_c, :].T   [N, E/8]  bf16   (stage-1 moving operand)
  ht  = Ht[:, n_c]     [E, N/8]  bf16   (stage-2 moving operand)
  stage1: tmpT[f,e] = sum_n X[n-tile].T @ hsT[n-tile]      (PE, N=512 free)
  msg[e,h] = tmpT.T @ theta ; scaled = edge_w * msg        -> AllGather
  stage2: aggT[h,n] = w_trans.T @ xiT + bias (rank-1)
          + sum_e scaled[e-tile].T @ ht[e-tile]            (PE, N=512 free)
  epilogue: fused leaky-relu / dropout mask / second lrelu  (DVE)
  conv0 only: PE-transpose X1T -> X1 tiles -> 2-chunk AllGather
  fc: fc_w.T @ XT (f32 matmul) + host-precomputed state term -> [1, N/8]

DMA layout notes: every dma_start costs ~0.6-2us of serial issue time on
the triggering engine, and a single InstDMACopy is already split across
all 16 SDMA engines, so transfers are consolidated into a few large DMAs
and spread across both HWDGE rings (sync + scalar). All small parameters
are packed host-side into one bf16 and one f32 tensor.
"""

import sys

if "/opt/trn_rl_repo" not in sys.path:
    sys.path.insert(0, "/opt/trn_rl_repo")

import numpy as np
import ml_dtypes

NCORES = 8
N, E, F = 8192, 4096, 128
E_SH = E // NCORES   # 512 edges per core
N_SH = N // NCORES   # 1024 nodes per core
NEG_SLOPE = 0.01
DROP_P = 0.5

# packed bf16 params layout (columns)
PB_TH = 0          # th0, th1         [128, 128] each
PB_WT = 256        # wt0, wt1
PB_IDN = 512       # identity
PB_XIT = 640       # xiT              [128, 1024]
PB_B = 1664        # b0, b1 on partition 0, 128 cols each
PB_W = 1920
# packed f32 params layout (columns)
PF_EW = 0          # ew0, ew1         [128, 4] each
PF_M2T = 8         # mask2T           [128, 1024]
PF_FCW = 1032      # fc_w[:128]       [128, 1]
PF_ST = 1033       # state term on partition 0, 1024 cols
PF_W = 2060

_CACHE = {}


def _build_nc():
    import concourse.bacc as bacc
    import concourse.mybir as mybir
    import concourse.tile as tile
    from concourse.tile import add_dep_helper

    bf16 = mybir.dt.bfloat16
    f32 = mybir.dt.float32
    Alu = mybir.AluOpType

    nc = bacc.Bacc("TRN2", target_bir_lowering=False, debug=False,
                   num_devices=NCORES)

    hsT_d = nc.dram_tensor("hsT", [N, E_SH], bf16, kind="ExternalInput")
    ht_d = nc.dram_tensor("ht", [E, N_SH], bf16, kind="ExternalInput")
    xbf_d = nc.dram_tensor("xbf", [N, F], bf16, kind="ExternalInput")
    pbf_d = nc.dram_tensor("pbf", [128, PB_W], bf16, kind="ExternalInput")
    pf32_d = nc.dram_tensor("pf32", [128, PF_W], f32, kind="ExternalInput")
    out_d = nc.dram_tensor("out", [1, N_SH], f32, kind="ExternalOutput")

    RG = [list(range(NCORES))]

    with tile.TileContext(nc) as tc:
        with (
            tc.tile_pool(name="sb", bufs=1) as sb,
            tc.tile_pool(name="sc2", bufs=2) as sc2,
            tc.tile_pool(name="ps_tmp", bufs=1, space="PSUM") as ps_tmp,
            tc.tile_pool(name="ps_agg", bufs=2, space="PSUM") as ps_agg,
            tc.tile_pool(name="ps_sm", bufs=2, space="PSUM") as ps_sm,
            tc.tile_pool(name="dram", bufs=1, space="DRAM") as dram,
        ):
            # ---- packed params (scalar ring) ----
            pbf = sb.tile([128, PB_W], bf16, tag="pbf")
            nc.scalar.dma_start(pbf[:], pbf_d[:, :])
            pf = sb.tile([128, PF_W], f32, tag="pf")
            nc.scalar.dma_start(pf[:], pf32_d[:, :])

            def th(conv):
                return pbf[:, PB_TH + conv * 128:PB_TH + (conv + 1) * 128]

            def wt(conv):
                return pbf[:, PB_WT + conv * 128:PB_WT + (conv + 1) * 128]

            idn = pbf[:, PB_IDN:PB_IDN + 128]
            xiT = pbf[:, PB_XIT:PB_XIT + 1024]

            def bias(conv):
                return pbf[0:1, PB_B + conv * 128:PB_B + (conv + 1) * 128]

            def ew(conv, ec):
                c0 = PF_EW + conv * 4 + ec
                return pf[:, c0:c0 + 1]

            m2T = pf[:, PF_M2T:PF_M2T + 1024]
            fcw = pf[:, PF_FCW:PF_FCW + 1]
            stT = pf[0:1, PF_ST:PF_ST + 1024]

            ones_sb = sb.tile([1, 512], bf16, tag="ones")
            nc.vector.memset(ones_sb[:], 1.0)

            # ---- big resident loads ----
            # x (conv0 stage-1 lhsT): 2 chunks [128, 32, 128] on sync ring
            xall = []
            x_r = xbf_d.ap().rearrange("(b p) h -> p b h", p=128)
            for i in range(2):
                xc = sb.tile([128, 32, F], bf16, tag=f"x{i}")
                nc.sync.dma_start(xc[:], x_r[:, i * 32:(i + 1) * 32, :])
                xall.append(xc)
            # hsT: 4 chunks [128, 16, 512] on sync ring
            hsT_t = []
            hsT_r = hsT_d.ap().rearrange("(b p) e -> p b e", p=128)
            for i in range(4):
                hc = sb.tile([128, 16, E_SH], bf16, tag=f"hsT{i}")
                nc.sync.dma_start(hc[:], hsT_r[:, i * 16:(i + 1) * 16, :])
                hsT_t.append(hc)

            ht_t = [None, None]
            ht_r = ht_d.ap().rearrange("(b p) n -> p b n", p=128)

            # collective bounce buffers
            agm_in = [dram.tile([E_SH, F], bf16, tag=f"agmi{i}",
                                name=f"agmi{i}") for i in range(2)]
            agm_out = [dram.tile([E, F], bf16, addr_space="Shared",
                                 tag=f"agmo{i}", name=f"agmo{i}")
                       for i in range(2)]
            agx_in = dram.tile([N_SH, F], bf16, tag="agxi")
            agx_out = [dram.tile([N // 2, F], bf16, addr_space="Shared",
                                 tag=f"agxo{j}", name=f"agxo{j}")
                       for j in range(2)]

            x1c = [None, None]   # gathered X1 chunks for conv1 stage 1
            x1t_tiles = [None, None]
            xT = [None, None]    # final-layer activations (f32)

            for conv in range(2):
                # ---------- stage 1: tmpT = X.T @ HsT ----------
                tmpT_ps = ps_tmp.tile([128, E_SH], f32, tag="tmpT")
                mm_last = None
                if conv == 0:
                    for nt in range(64):
                        mm_last = nc.tensor.matmul(
                            tmpT_ps[:], xall[nt // 32][:, nt % 32, :],
                            hsT_t[nt // 16][:, nt % 16, :],
                            start=(nt == 0), stop=(nt == 63))
                else:
                    # consume gathered X1 chunk 0 first, then chunk 1
                    cnt = 0
                    for j in range(2):
                        for b in range(32):
                            nt = (b // 4) * 8 + j * 4 + (b % 4)
                            mm_last = nc.tensor.matmul(
                                tmpT_ps[:], x1c[j][:, b, :],
                                hsT_t[nt // 16][:, nt % 16, :],
                                start=(cnt == 0), stop=(cnt == 63))
                            cnt += 1

                if conv == 0:
                    # ht loads deferred behind stage 1 so they don't steal
                    # HBM bandwidth from the critical hsT/x stream; they
                    # fill the AllGather window instead.
                    for i in range(2):
                        hc = sb.tile([128, 16, N_SH], bf16, tag=f"ht{i}")
                        dma = nc.scalar.dma_start(
                            hc[:], ht_r[:, i * 16:(i + 1) * 16, :])
                        add_dep_helper(dma.ins, mm_last.ins, sync=True,
                                       reason="defer ht behind stage1")
                        ht_t[i] = hc

                tmpT_bf = sb.tile([128, E_SH], bf16, tag=f"tmpTbf{conv}")
                nc.vector.tensor_copy(tmpT_bf[:], tmpT_ps[:])

                # ---------- msg = tmpT.T @ theta, scaled by edge_w ----------
                msg_sb = sb.tile([128, 4, F], bf16, tag="msg")
                for ec in range(4):
                    mps = ps_sm.tile([128, F], f32, tag="msg", bufs=2)
                    nc.tensor.matmul(
                        mps[:], tmpT_bf[:, ec * 128:(ec + 1) * 128],
                        th(conv), start=True, stop=True)
                    nc.vector.tensor_scalar(
                        msg_sb[:, ec, :], mps[:], ew(conv, ec), None, Alu.mult)
                nc.sync.dma_start(
                    agm_in[conv].rearrange("(c p) h -> p c h", p=128),
                    msg_sb[:])

                nc.gpsimd.collective_compute(
                    "AllGather", Alu.bypass, replica_groups=RG,
                    ins=[agm_in[conv][:]], outs=[agm_out[conv][:]])

                sc_t = sb.tile([128, 32, F], bf16, tag="sc")
                nc.scalar.dma_start(
                    sc_t[:], agm_out[conv].rearrange("(b p) h -> p b h",
                                                     p=128))

                # ---------- stage 2: aggT = wT@xiT + b + scaled.T @ Ht ----------
                for nb in range(2):
                    agg = ps_agg.tile([128, 512], f32, tag="agg")
                    nc.tensor.matmul(
                        agg[:], wt(conv), xiT[:, nb * 512:(nb + 1) * 512],
                        start=True, stop=False)
                    nc.tensor.matmul(
                        agg[:], bias(conv), ones_sb[:],
                        start=False, stop=False)
                    for et in range(32):
                        nc.tensor.matmul(
                            agg[:], sc_t[:, et, :],
                            ht_t[et // 16][:, et % 16, nb * 512:(nb + 1) * 512],
                            start=False, stop=(et == 31))

                    if conv == 0:
                        # X1T = lrelu(agg) * dropout_mask   (bf16)
                        sl = sc2.tile([128, 512], f32, tag="sl")
                        nc.vector.tensor_scalar(
                            sl[:], agg[:], NEG_SLOPE, None, Alu.mult)
                        lr = sc2.tile([128, 512], f32, tag="lr")
                        nc.vector.tensor_tensor(lr[:], agg[:], sl[:], Alu.max)
                        x1t = sb.tile([128, 512], bf16, tag=f"x1t{nb}")
                        nc.vector.tensor_tensor(
                            x1t[:], lr[:], m2T[:, nb * 512:(nb + 1) * 512],
                            Alu.mult)
                        x1t_tiles[nb] = x1t
                    else:
                        # X = lrelu(lrelu(agg)) = max(agg, 1e-4*agg)  (f32)
                        sl = sc2.tile([128, 512], f32, tag="sl")
                        nc.vector.tensor_scalar(
                            sl[:], agg[:], NEG_SLOPE * NEG_SLOPE, None,
                            Alu.mult)
                        t = sb.tile([128, 512], f32, tag=f"xT{nb}")
                        nc.vector.tensor_tensor(t[:], agg[:], sl[:], Alu.max)
                        xT[nb] = t

                if conv == 0:
                    # transpose X1T -> node-major X1, bounce to DRAM,
                    # then 2-chunk AllGather so conv1 stage 1 can start on
                    # chunk 0 while chunk 1 is still in flight.
                    x1loc = sb.tile([128, 8, F], bf16, tag="x1loc")
                    for nt in range(8):
                        tps = ps_sm.tile([128, 128], bf16, tag="tr", bufs=2)
                        j = nt % 4
                        nc.tensor.transpose(
                            tps[:],
                            x1t_tiles[nt // 4][:, j * 128:(j + 1) * 128],
                            idn)
                        nc.vector.tensor_copy(x1loc[:, nt, :], tps[:])
                    nc.sync.dma_start(
                        agx_in.rearrange("(c p) h -> p c h", p=128),
                        x1loc[:])
                    for j in range(2):
                        nc.gpsimd.collective_compute(
                            "AllGather", Alu.bypass, replica_groups=RG,
                            ins=[agx_in[j * 512:(j + 1) * 512, :]],
                            outs=[agx_out[j][:]])
                        xc = sb.tile([128, 32, F], bf16, tag=f"x1c{j}")
                        nc.scalar.dma_start(
                            xc[:], agx_out[j].rearrange("(b p) h -> p b h",
                                                        p=128))
                        x1c[j] = xc

            # ---------- fc ----------
            osb = sc2.tile([1, 1024], f32, tag="osb")
            for nb in range(2):
                fps = ps_sm.tile([1, 512], f32, tag="fc", bufs=1)
                nc.tensor.matmul(fps[:], fcw, xT[nb][:], start=True, stop=True)
                nc.vector.tensor_tensor(
                    osb[:, nb * 512:(nb + 1) * 512], fps[:],
                    stT[:, nb * 512:(nb + 1) * 512], Alu.add)
            nc.sync.dma_start(out_d[0:1, :], osb[:])

    nc.compile()
    return nc


def _get_nc():
    if "nc" not in _CACHE:
        _CACHE["nc"] = _build_nc()
    return _CACHE["nc"]


def _dropout_mask2():
    """2.0 * bernoulli(key(42), 0.5, (N, F)) exactly as the reference."""
    import jax
    cpu = jax.devices("cpu")[0]
    with jax.default_device(cpu):
        keep = jax.random.bernoulli(jax.random.key(42), 1.0 - DROP_P, (N, F))
        return np.asarray(keep).astype(np.float32) * (1.0 / (1.0 - DROP_P))


def prepare_in_maps(xi, x, Ht, Hs, state,
                    w_trans0, theta0, edge_w0, bias0,
                    w_trans1, theta1, edge_w1, bias1,
                    fc_w, fc_b):
    bf = ml_dtypes.bfloat16
    mask2 = _dropout_mask2()

    xbf = np.ascontiguousarray(x, np.float32).astype(bf)
    fcw32 = np.asarray(fc_w, np.float32)
    fcw_last = float(fcw32[F, 0])
    fcb = float(np.asarray(fc_b, np.float32)[0])

    Hs32 = np.asarray(Hs, np.float32)
    Ht32 = np.asarray(Ht, np.float32)
    xi32 = np.asarray(xi, np.float32)
    st32 = np.asarray(state, np.float32)
    th = [np.asarray(theta0, np.float32), np.asarray(theta1, np.float32)]
    wtr = [np.asarray(w_trans0, np.float32), np.asarray(w_trans1, np.float32)]
    bs = [np.asarray(bias0, np.float32), np.asarray(bias1, np.float32)]
    ews = [np.asarray(edge_w0, np.float32), np.asarray(edge_w1, np.float32)]

    in_maps = []
    for c in range(NCORES):
        e0, e1 = c * E_SH, (c + 1) * E_SH
        n0, n1 = c * N_SH, (c + 1) * N_SH

        pbf = np.zeros((128, PB_W), np.float32)
        pbf[:, PB_TH:PB_TH + 128] = th[0]
        pbf[:, PB_TH + 128:PB_TH + 256] = th[1]
        pbf[:, PB_WT:PB_WT + 128] = wtr[0]
        pbf[:, PB_WT + 128:PB_WT + 256] = wtr[1]
        pbf[:, PB_IDN:PB_IDN + 128] = np.eye(F)
        pbf[:, PB_XIT:PB_XIT + 1024] = xi32[n0:n1, :].T
        pbf[0, PB_B:PB_B + 128] = bs[0]
        pbf[0, PB_B + 128:PB_B + 256] = bs[1]

        pf = np.zeros((128, PF_W), np.float32)
        pf[:, PF_EW:PF_EW + 4] = ews[0][e0:e1].reshape(4, 128).T
        pf[:, PF_EW + 4:PF_EW + 8] = ews[1][e0:e1].reshape(4, 128).T
        pf[:, PF_M2T:PF_M2T + 1024] = mask2[n0:n1, :].T
        pf[:, PF_FCW:PF_FCW + 1] = fcw32[:F, :]
        pf[0, PF_ST:PF_ST + 1024] = st32[n0:n1, 0] * fcw_last + fcb

        in_maps.append({
            "hsT": np.ascontiguousarray(Hs32[e0:e1, :].T).astype(bf),
            "ht": np.ascontiguousarray(Ht32[:, n0:n1]).astype(bf),
            "xbf": xbf,
            "pbf": pbf.astype(bf),
            "pf32": pf,
        })
    return in_maps


def kernel(xi, x, Ht, Hs, state,
           w_trans0, theta0, edge_w0, bias0,
           w_trans1, theta1, edge_w1, bias1,
           fc_w, fc_b, _trace=False):
    from concourse.bass_utils import run_bass_kernel_spmd

    nc = _get_nc()
    in_maps = prepare_in_maps(
        xi, x, Ht, Hs, state,
        w_trans0, theta0, edge_w0, bias0,
        w_trans1, theta1, edge_w1, bias1,
        fc_w, fc_b)
    res = run_bass_kernel_spmd(
        nc, in_maps, core_ids=list(range(NCORES)), trace=_trace)
    if _trace:
        _CACHE["last_results"] = res
    out = np.concatenate(
        [res.results[c]["out"].reshape(N_SH) for c in range(NCORES)])
    return out.reshape(N, 1).astype(np.float32)
